# revision 11
# baseline (speedup 1.0000x reference)
"""Trainium2 Bass kernel for the DGRU problem (nn_DGRU_36429912605229).

Strategy (pure data parallel, 8 cores x 32 batch):
  - Host: fold the input-side math (f = Wf s + bf, alpha = sigmoid(Wa f + ba),
    se = s + alpha*f) into an extended 15-feature vector
        u = [s(6), 1, m, alpha*s(6), alpha]
    so that every gate pre-activation is one K=15 matmul:
        pre_G = W_G' @ u,   W_G' = [W | b | (-BIG if z) | W@Wf | W@bf]
    The mask enters the z gate additively (-BIG * m -> sigmoid ~= 0 ->
    h_new == h exactly), and the "take h at t=len-1" gather is folded into the
    mask by freezing h for all t > idx (mask |= t > idx).  alpha itself is
    computed on device; only layout packing happens on host.
  - Device phase A: compute alpha and u (block-diagonal matmul tricks for the
    per-(b,t)-scalar broadcast), write u to DRAM in per-16-step blocks.
  - Device phase B: sequential GRU scan.  Per 16-step block, one K=15 matmul
    per gate computes the x-side preacts straight into PSUM; per step the
    recurrent matmuls (Uz,Ur,Uh) accumulate into the same PSUM columns, ACT
    applies sigmoid/tanh, DVE does the elementwise update:
        zeff = sig(pz + Uz h); r = sig(pr + Ur h)
        [A|rh] = [zeff|r] * [g|h]          (one fused DVE op)
        negBv  = (zeff - 1) * h            (one scalar_tensor_tensor op)
        htil   = tanh(ph + Uh rh)
        h      = A*htil - negBv
  - Device phase C: h / max(||h||, 1e-12) via sum-of-squares matmul with a
    ones vector, rsqrt = exp(-0.5*ln(ss)), PE broadcast, multiply.
"""

import numpy as np

import concourse.bass as bass
import concourse.bacc as bacc
import concourse.mybir as mybir
from concourse import tile
from concourse.bass_utils import run_bass_kernel_spmd
from concourse.bass_interp import get_hw_module

F32 = mybir.dt.float32
AF = mybir.ActivationFunctionType
OP = mybir.AluOpType

B, L, IN_DIM, H = 256, 2048, 6, 128
NCORES = 8
BSH = B // NCORES                 # 32 batch per core
N = BSH * L                       # 65536 (b,t) pairs per core, t-major
T_BLK = 16                        # timesteps per PSUM block
NBLK = L // T_BLK                 # 128 blocks
BODY_BLKS = 4                     # blocks per loop iteration
NITER = NBLK // BODY_BLKS         # 32 loop iterations
CHUNK = T_BLK * BSH               # 512 columns per block
SLOTS_PER_CHUNK = 6               # phase-A slots per [128, 512] tile
NCHUNK_A = (NBLK + SLOTS_PER_CHUNK - 1) // SLOTS_PER_CHUNK  # 22
BIG = 30000.0

_CACHED = {}


def _build_module():
    """Build (once) the Bass module shared by all cores."""
    if "nc" in _CACHED:
        return _CACHED["nc"]

    nc = bacc.Bacc("TRN2", target_bir_lowering=False, debug=False,
                   num_devices=NCORES)

    # ---- DRAM tensors (per-core data arrives via in_maps) ----
    uin = nc.dram_tensor("uin", [NCHUNK_A, 128, CHUNK], F32,
                         kind="ExternalInput").ap()
    wp = nc.dram_tensor("wp", [16, 4, 128], F32, kind="ExternalInput").ap()
    bd1 = nc.dram_tensor("bd1", [128, 128], F32, kind="ExternalInput").ap()
    bd2 = nc.dram_tensor("bd2", [128, 128], F32, kind="ExternalInput").ap()
    uzt = nc.dram_tensor("uzt", [128, 128], F32, kind="ExternalInput").ap()
    urt = nc.dram_tensor("urt", [128, 128], F32, kind="ExternalInput").ap()
    uht = nc.dram_tensor("uht", [128, 128], F32, kind="ExternalInput").ap()
    hout = nc.dram_tensor("hout", [128, BSH], F32, kind="ExternalOutput").ap()
    # intermediate: u blocks, [dim0 = 16*blk_in_iter + row, dim1 = iter, cols]
    ufin = nc.dram_tensor("ufin", [16 * BODY_BLKS, NITER, CHUNK], F32,
                          kind="Internal").ap()

    with tile.TileContext(nc) as tc:
        # ======== weights to SBUF ========
        with tc.tile_pool(name="wpool", bufs=1) as wpool:
            wp_sb = wpool.tile([16, 4, 128], F32)      # 4 gate lhsTs
            bd1_sb = wpool.tile([128, 128], F32)
            bd2_sb = wpool.tile([128, 128], F32)
            uzt_sb = wpool.tile([128, 128], F32)
            urt_sb = wpool.tile([128, 128], F32)
            uht_sb = wpool.tile([128, 128], F32)
            ones_col = wpool.tile([128, 1], F32)       # for sum of squares
            ones_row = wpool.tile([1, 128], F32)       # for broadcast
            nc.sync.dma_start(wp_sb[:, :, :], wp[:, :, :])
            nc.sync.dma_start(bd1_sb[:, :], bd1[:, :])
            nc.sync.dma_start(bd2_sb[:, :], bd2[:, :])
            nc.sync.dma_start(uzt_sb[:, :], uzt[:, :])
            nc.sync.dma_start(urt_sb[:, :], urt[:, :])
            nc.sync.dma_start(uht_sb[:, :], uht[:, :])
            nc.vector.memset(ones_col[:, :], 1.0)
            nc.vector.memset(ones_row[:, :], 1.0)

            # ======== phase A: build u (alpha folding) ========
            with (
                tc.tile_pool(name="pa_sbuf", bufs=3) as pa,
                tc.tile_pool(name="pa_out", bufs=3) as pa_out,
                tc.tile_pool(name="pa_psum", bufs=2,
                             space=bass.MemorySpace.PSUM) as pap,
                tc.tile_pool(name="pa_psum2", bufs=2,
                             space=bass.MemorySpace.PSUM) as pap2,
            ):
                for k in range(NCHUNK_A):
                    uch = pa.tile([128, CHUNK], F32, tag="uch")
                    nc.sync.dma_start(uch[:, :], uin[k, :, :])
                    psA = pap.tile([128, CHUNK], F32, tag="psA")
                    nc.tensor.matmul(psA[:, :], bd1_sb[:, :], uch[:, :],
                                     start=True, stop=True)
                    # sigmoid(alpha_pre) for the 6 slots -> rows 96..101
                    nc.scalar.activation(uch[96:102, :], psA[96:102, :],
                                         AF.Sigmoid)
                    psB = pap2.tile([128, CHUNK], F32, tag="psB")
                    nc.tensor.matmul(psB[:, :], bd2_sb[:, :], uch[:, :],
                                     start=True, stop=True)
                    ufc = pa_out.tile([128, CHUNK], F32, tag="ufc")
                    nc.vector.tensor_tensor(ufc[:, :], uch[:, :], psB[:, :],
                                            op=OP.mult)
                    for q in range(SLOTS_PER_CHUNK):
                        gb = k * SLOTS_PER_CHUNK + q   # global block index
                        if gb >= NBLK:
                            break
                        it, bb = gb // BODY_BLKS, gb % BODY_BLKS
                        nc.sync.dma_start(ufin[16 * bb:16 * bb + 16, it, :],
                                          ufc[16 * q:16 * q + 16, :])

            # ======== phase B: the scan ========
            with tc.tile_pool(name="gh_pool", bufs=1) as ghp:
              with (
                tc.tile_pool(name="ub_pool", bufs=1) as ubp,
                tc.tile_pool(name="st_pool", bufs=1) as stp,
                tc.tile_pool(name="ps_pool", bufs=1,
                             space=bass.MemorySpace.PSUM) as psp,
              ):
                gh = [ghp.tile([128, T_BLK * 64], F32, tag=f"gh{b}", name=f"gh{b}")
                      for b in range(BODY_BLKS)]
                # psum tiles: [gate][parity]
                ps = [[psp.tile([128, CHUNK], F32, tag=f"ps{g}_{p}", name=f"ps{g}_{p}")
                       for p in range(2)] for g in range(4)]
                ubt = [ubp.tile([16, 1, CHUNK], F32, tag=f"ub{b}", name=f"ub{b}")
                       for b in range(BODY_BLKS)]
                # state/update tiles, one set per step parity to allow overlap
                NSET = 4
                zr_t = [stp.tile([128, 64], F32, tag=f"zr{j}", name=f"zr{j}") for j in range(NSET)]
                x2_t = [stp.tile([128, 64], F32, tag=f"x2{j}", name=f"x2{j}") for j in range(NSET)]
                nb_t = [stp.tile([128, 32], F32, tag=f"nb{j}", name=f"nb{j}") for j in range(NSET)]
                ht_t = [stp.tile([128, 32], F32, tag=f"ht{j}", name=f"ht{j}") for j in range(NSET)]
                d_t = [stp.tile([128, 32], F32, tag=f"d{j}", name=f"d{j}") for j in range(NSET)]

                # h0 = 0
                nc.vector.memset(gh[0][:, 32:64], 0.0)

                def h_slot(b, tl):
                    """AP of the h state entering step (b, tl)."""
                    return gh[b][:, 64 * tl + 32: 64 * tl + 64]

                def g_slot(b, tl):
                    return gh[b][:, 64 * tl: 64 * tl + 32]

                with tc.For_i(0, NITER, 1,
                              hint_engines=(mybir.EngineType.PE,
                                            mybir.EngineType.DVE,
                                            mybir.EngineType.Activation,
                                            mybir.EngineType.SP,
                                            mybir.EngineType.Pool)) as it:
                    for b in range(BODY_BLKS):
                        p = b % 2
                        # stream u block
                        nc.sync.dma_start(ubt[b][:, :, :],
                                          ufin[16 * b:16 * b + 16,
                                               bass.ds(it, 1), :])
                        ub = ubt[b]
                        # x-side preacts for the whole block
                        for g in range(4):
                            nc.tensor.matmul(ps[g][p][:, :],
                                             wp_sb[0:15, g, :], ub[0:15, 0, :],
                                             start=True, stop=True)
                        # g = sigmoid -> strided into gh slots
                        gview = gh[b][:, :].rearrange(
                            "p (t c) -> p t c", c=64)[:, :, 0:32]
                        pview = ps[0][p][:, :].rearrange(
                            "p (t c) -> p t c", c=32)
                        nc.scalar.activation(gview, pview, AF.Sigmoid)

                        for tl in range(T_BLK):
                            j = tl % NSET
                            h = h_slot(b, tl)
                            cs = slice(32 * tl, 32 * tl + 32)
                            zr, x2, nb, ht, d = (zr_t[j], x2_t[j], nb_t[j],
                                                 ht_t[j], d_t[j])
                            nc.tensor.matmul(ps[1][p][:, cs], uzt_sb[:, :], h,
                                             start=False, stop=False,
                                             skip_group_check=True)
                            nc.tensor.matmul(ps[2][p][:, cs], urt_sb[:, :], h,
                                             start=False, stop=False,
                                             skip_group_check=True)
                            nc.scalar.activation(zr[:, 0:32], ps[1][p][:, cs],
                                                 AF.Sigmoid)
                            nc.scalar.activation(zr[:, 32:64], ps[2][p][:, cs],
                                                 AF.Sigmoid)
                            # negBv = (zeff - 1) * h
                            nc.vector.scalar_tensor_tensor(
                                nb[:, :], zr[:, 0:32], 1.0, h,
                                op0=OP.subtract, op1=OP.mult)
                            # [A | rh] = [zeff | r] * [g | h]
                            nc.vector.tensor_tensor(
                                x2[:, :], zr[:, :], gh[b][:, 64 * tl:64 * tl + 64],
                                op=OP.mult)
                            nc.tensor.matmul(ps[3][p][:, cs], uht_sb[:, :],
                                             x2[:, 32:64],
                                             start=False, stop=False,
                                             skip_group_check=True)
                            nc.scalar.activation(ht[:, :], ps[3][p][:, cs],
                                                 AF.Tanh)
                            nc.vector.tensor_tensor(d[:, :], x2[:, 0:32],
                                                    ht[:, :], op=OP.mult)
                            # h_next = D - negBv
                            if tl < T_BLK - 1:
                                hn = h_slot(b, tl + 1)
                            elif b < BODY_BLKS - 1:
                                hn = h_slot(b + 1, 0)
                            else:
                                hn = h_slot(0, 0)
                            nc.vector.tensor_tensor(hn, d[:, :], nb[:, :],
                                                    op=OP.subtract)

              # ======== phase C: normalize (after psum pool closes) ========
              with tc.tile_pool(name="pc", bufs=1) as pc, \
                   tc.tile_pool(name="pcp", bufs=1,
                                space=bass.MemorySpace.PSUM) as pcp:
                hfin = gh[0][:, 32:64]
                sq = pc.tile([128, BSH], F32)
                nc.vector.tensor_tensor(sq[:, :], hfin, hfin, op=OP.mult)
                ssp = pcp.tile([1, BSH], F32)
                nc.tensor.matmul(ssp[:, :], ones_col[:, :], sq[:, :],
                                 start=True, stop=True)
                ssc = pc.tile([1, BSH], F32)
                nc.vector.tensor_scalar(ssc[:, :], ssp[:, :], 1e-24, None,
                                        op0=OP.max)
                lns = pc.tile([1, BSH], F32)
                nc.scalar.activation(lns[:, :], ssc[:, :], AF.Ln)
                rsq = pc.tile([1, BSH], F32)
                nc.scalar.activation(rsq[:, :], lns[:, :], AF.Exp,
                                     scale=-0.5)
                bcp = pcp.tile([128, BSH], F32)
                nc.tensor.matmul(bcp[:, :], ones_row[:, :], rsq[:, :],
                                 start=True, stop=True)
                hn_sb = pc.tile([128, BSH], F32)
                nc.vector.tensor_tensor(hn_sb[:, :], hfin, bcp[:, :],
                                        op=OP.mult)
                nc.sync.dma_start(hout[:, :], hn_sb[:, :])

    nc.compile()
    nc.m = get_hw_module(nc.m)
    _CACHED["nc"] = nc
    return nc


def _host_prep(s, lens, mask, Wf, bf, Wa, ba, Wg, bg, Wz, bz, Wr, br,
               Wh, bh, Uz, Ur, Uh):
    """Build per-core input maps."""
    s = np.asarray(s, np.float32)
    lens = np.asarray(lens)
    mask = np.asarray(mask, bool)
    f32 = lambda x: np.asarray(x, np.float32)
    Wf, bf, Wa, ba = f32(Wf), f32(bf), f32(Wa), f32(ba)
    Wg, bg, Wz, bz = f32(Wg), f32(bg), f32(Wz), f32(bz)
    Wr, br, Wh, bh = f32(Wr), f32(br), f32(Wh), f32(bh)
    Uz, Ur, Uh = f32(Uz), f32(Ur), f32(Uh)

    idx = np.maximum(lens.astype(np.int64), 1) - 1
    mp = (mask | (np.arange(L)[None, :] > idx[:, None])).astype(np.float32)

    # gate weights (lhsT layout [15 rows, 128 cols]), padded to 16 rows
    def gate_w(W, bvec, is_z):
        rows = np.zeros((16, H), np.float32)
        rows[0:6] = W.T
        rows[6] = bvec
        rows[7] = -BIG if is_z else 0.0
        rows[8:14] = (W @ Wf).T
        rows[14] = W @ bf
        return rows

    wp = np.ascontiguousarray(np.stack(
        [gate_w(Wg, bg, False), gate_w(Wz, bz, True),
         gate_w(Wr, br, False), gate_w(Wh, bh, False)]).transpose(1, 0, 2))

    waWf = (Wa @ Wf)[0]                    # [6]
    wac = float((Wa @ bf + ba)[0])

    # phase-A block-diagonal matrices
    bd1 = np.zeros((128, 128), np.float32)
    bd2 = np.zeros((128, 128), np.float32)
    for q in range(SLOTS_PER_CHUNK):
        r0 = 16 * q
        bd1[r0:r0 + 6, 96 + q] = waWf
        bd1[r0 + 6, 96 + q] = wac
        bd2[r0 + 6, r0:r0 + 8] = 1.0       # rows 0..7 multiplier 1
        bd2[96 + q, r0 + 8:r0 + 15] = 1.0  # rows 8..14 multiplier alpha

    in_maps = []
    for c in range(NCORES):
        sc = s[BSH * c:BSH * (c + 1)]              # [32, L, 6]
        mc = mp[BSH * c:BSH * (c + 1)]             # [32, L]
        S_tm = np.ascontiguousarray(sc.transpose(1, 0, 2)).reshape(N, 6)
        M_tm = np.ascontiguousarray(mc.T).reshape(N)
        # slots [128, 16, CHUNK]
        u15 = np.zeros((NBLK, 16, CHUNK), np.float32)
        St = S_tm.reshape(NBLK, CHUNK, 6).transpose(0, 2, 1)  # [128,6,512]
        u15[:, 0:6] = St
        u15[:, 6] = 1.0
        u15[:, 7] = M_tm.reshape(NBLK, CHUNK)
        u15[:, 8:14] = St
        u15[:, 14] = 1.0
        uin = np.zeros((NCHUNK_A, 128, CHUNK), np.float32)
        for k in range(NCHUNK_A):
            nslot = min(SLOTS_PER_CHUNK, NBLK - k * SLOTS_PER_CHUNK)
            blkrange = u15[k * SLOTS_PER_CHUNK:k * SLOTS_PER_CHUNK + nslot]
            uin[k, :16 * nslot] = blkrange.reshape(16 * nslot, CHUNK)
        in_maps.append({
            "uin": uin,
            "wp": wp,
            "bd1": bd1,
            "bd2": bd2,
            "uzt": np.ascontiguousarray(Uz.T),
            "urt": np.ascontiguousarray(Ur.T),
            "uht": np.ascontiguousarray(Uh.T),
        })
    return in_maps


def kernel(**inputs) -> np.ndarray:
    nc = _build_module()
    in_maps = _host_prep(**inputs)
    res = run_bass_kernel_spmd(nc, in_maps, core_ids=list(range(NCORES)))
    out = np.empty((B, H), np.float32)
    for c in range(NCORES):
        out[BSH * c:BSH * (c + 1)] = res.results[c]["hout"].T
    return out


if __name__ == "__main__":
    import reference
    inputs = {k: np.asarray(v) for k, v in reference.setup_inputs().items()}
    got = kernel(**inputs)
    print("kernel output", got.shape, got.dtype)
